# revision 4
# baseline (speedup 1.0000x reference)
"""MLA (CustomLlamaMLAForInfer) Trainium2 Bass kernel.

Sharding: tensor-parallel over heads across 8 NeuronCores. Core c owns
kv-head c and q-heads [4c, 4c+4). Every core sees the full token stream
(B*S = 4096 tokens); o_proj is computed against the core's 512
head-dims, producing a partial [4096, 4096] output that the host sums
across the 8 cores.

Device program phases (single SPMD program, per-core weights differ):
  1a. qT = Wq_shard @ hidden.T   (rope + 1/sqrt(d) folded in at evict)
  1b. c_kvT = Wdk @ hidden.T ; krT = Wkr_shard @ hidden.T (rope at evict)
  2.  k_c / v from c_kvT via Wupk/Wupv shards; assemble kT_full, v_tok
  3.  causal attention per (batch, q-head): scores_T = kT.T@qT blocks,
      exp (no max-sub needed: |scores| < ~6), mask diag blocks,
      out_T[d,q] += v_tok.T @ p_T, sums via ones-matmul, normalize
  4.  partial o_proj: out[tok, hid] += attn_T.T @ WoT_shard

All matmuls run as float32r (fp22 mantissa, 1 PE pass).
"""

import numpy as np

HIDDEN = 4096
N_HEADS = 32
KV_HEADS = 8
HEAD_DIM = 128
LOW_RANK = 64
TOP_K_ROPE = 32
ROPE_THETA = 10000.0
B, S = 2, 2048
NCORES = 8
HPC = N_HEADS // NCORES          # q heads per core = 4
QR = HPC * HEAD_DIM              # q rows per core = 512
CD = LOW_RANK * KV_HEADS         # latent dim = 512
KRR = 2 * TOP_K_ROPE             # rope rows per kv head = 64


def _rope_tables(seq_len):
    inv = 1.0 / (ROPE_THETA ** (np.arange(0, HEAD_DIM, 2, dtype=np.float32) / HEAD_DIM))
    pos = np.arange(seq_len, dtype=np.float32)
    fr = np.outer(pos, inv)
    emb = np.concatenate([fr, fr], axis=-1)          # [S, 128]
    return (np.cos(emb).T.astype(np.float32),        # [128, S]
            np.sin(emb).T.astype(np.float32))


def build_program(Bv=B, Sv=S, TB=512, QB=512):
    """Build the SPMD Bass program. TB = proj token-block, QB = attention
    q-block (both <= 512, the fp32 moving-operand limit)."""
    from concourse import bacc, tile, mybir
    import concourse.bass as bass

    f32 = mybir.dt.float32
    F32R = mybir.dt.float32r
    MS = bass.MemorySpace
    EXP = mybir.ActivationFunctionType.Exp

    NT = Bv * Sv                 # total tokens
    HT = HIDDEN // 128           # hidden tiles = 32
    NTB = NT // TB               # proj token blocks
    NQB = Sv // QB               # q blocks per batch
    NJ = QB // 128               # diagonal mask variants
    NKT_B = Sv // 128            # k tiles per batch
    QT = QR // 128               # q-head tiles per core = 4
    LT = CD // 128               # latent tiles = 4

    nc = bacc.Bacc("TRN2", target_bir_lowering=False, debug=False,
                   num_devices=NCORES)

    def din(name, shape):
        return nc.dram_tensor(name, shape, f32, kind="ExternalInput").ap()

    hidT = din("hidT", [HIDDEN, NT])
    wq = din("wq_t", [HIDDEN, QR])
    wkd = din("wkd_t", [HIDDEN, CD + KRR])
    wupk = din("wupk_t", [CD, KRR])
    wupv = din("wupv_t", [CD, HEAD_DIM])
    wo = din("wo_t", [QR, HIDDEN])
    qcos = din("qcos", [128, NT])
    qsin = din("qsin", [128, NT])
    kcos = din("kcos", [KRR, NT])
    ksin = din("ksin", [KRR, NT])
    masks = din("masks", [128, NJ, QB])
    onesd = din("ones", [128, 1])
    outp = nc.dram_tensor("out_part", [NT, HIDDEN], f32, kind="ExternalOutput").ap()
    qT_s = nc.dram_tensor("qT_s", [QT, 128, NT], f32).ap()
    ckv_s = nc.dram_tensor("ckv_s", [LT, 128, NT], f32).ap()

    with tile.TileContext(nc) as tc:
        with tc.tile_pool(name="persist", bufs=1) as pers:
            kT_full = pers.tile([128, NT], F32R, tag="kT")
            v_tok = pers.tile([128, NT // 128, HEAD_DIM], F32R, tag="vtok")

            # ---------------- phase 1: projections of hidden ----------------
            with tc.tile_pool(name="cos", bufs=1) as cp:
                qcos_sb = cp.tile([128, NT], f32, tag="qc")
                qsin_sb = cp.tile([128, NT], f32, tag="qs")
                kcos_sb = cp.tile([KRR, NT], f32, tag="kc")
                ksin_sb = cp.tile([KRR, NT], f32, tag="ks")
                nc.sync.dma_start(qcos_sb[:], qcos)
                nc.sync.dma_start(qsin_sb[:], qsin)
                nc.sync.dma_start(kcos_sb[:], kcos)
                nc.sync.dma_start(ksin_sb[:], ksin)

                # ---- pass A: q projection (+rope, +1/sqrt(d) via tables) ----
                with tc.tile_pool(name="wqp", bufs=1) as wqp, \
                     tc.tile_pool(name="hidA", bufs=8) as hpA, \
                     tc.tile_pool(name="stA", bufs=2) as stA, \
                     tc.tile_pool(name="psA", bufs=8, space=MS.PSUM) as ppA:
                    wq_sb = wqp.tile([128, HT, QR], F32R)
                    nc.sync.dma_start(wq_sb[:], wq.rearrange("(t p) w -> p t w", p=128).bitcast(F32R))
                    for blk in range(NTB):
                        c0, c1 = blk * TB, (blk + 1) * TB
                        qps = [ppA.tile([128, TB], f32, tag="qps", name=f"qps{_m}") for _m in range(QT)]
                        for t in range(HT):
                            ht = hpA.tile([128, TB], F32R, tag="hid")
                            nc.sync.dma_start(ht[:], hidT[t * 128:(t + 1) * 128, c0:c1].bitcast(F32R))
                            for m in range(QT):
                                nc.tensor.matmul(
                                    qps[m][:],
                                    wq_sb[:, t, m * 128:(m + 1) * 128],
                                    ht[:],
                                    start=(t == 0), stop=(t == HT - 1))
                        for m in range(QT):
                            raw = stA.tile([128, TB], f32, tag="raw")
                            nc.scalar.copy(raw[:], qps[m][:])
                            rot = stA.tile([128, TB], f32, tag="rot")
                            nc.sync.dma_start(rot[0:64, :], raw[64:128, :])
                            nc.sync.dma_start(rot[64:128, :], raw[0:64, :])
                            qsb = stA.tile([128, TB], f32, tag="qsb")
                            nc.vector.tensor_mul(qsb[:], raw[:], qcos_sb[:, c0:c1])
                            nc.vector.tensor_mul(rot[:], rot[:], qsin_sb[:, c0:c1])
                            nc.vector.tensor_add(qsb[:], qsb[:], rot[:])
                            nc.sync.dma_start(qT_s[m, :, c0:c1], qsb[:])

                # ---- pass B: c_kv (latent) + k_rope projections ----
                with tc.tile_pool(name="wkdp", bufs=1) as wkdp, \
                     tc.tile_pool(name="hidB", bufs=8) as hpB, \
                     tc.tile_pool(name="stB", bufs=2) as stB, \
                     tc.tile_pool(name="psB", bufs=6, space=MS.PSUM) as ppB, \
                     tc.tile_pool(name="psBk", bufs=2, space=MS.PSUM) as ppBk:
                    wkd_sb = wkdp.tile([128, HT, CD + KRR], F32R)
                    nc.sync.dma_start(wkd_sb[:], wkd.rearrange("(t p) w -> p t w", p=128).bitcast(F32R))
                    for blk in range(NTB):
                        c0, c1 = blk * TB, (blk + 1) * TB
                        dps = [ppB.tile([128, TB], f32, tag="dps", name=f"dps{_m}") for _m in range(LT)]
                        krp = ppBk.tile([KRR, TB], f32, tag="krp")
                        for t in range(HT):
                            ht = hpB.tile([128, TB], F32R, tag="hid")
                            nc.sync.dma_start(ht[:], hidT[t * 128:(t + 1) * 128, c0:c1].bitcast(F32R))
                            for m in range(LT):
                                nc.tensor.matmul(
                                    dps[m][:],
                                    wkd_sb[:, t, m * 128:(m + 1) * 128],
                                    ht[:],
                                    start=(t == 0), stop=(t == HT - 1))
                            nc.tensor.matmul(
                                krp[:],
                                wkd_sb[:, t, CD:CD + KRR],
                                ht[:],
                                start=(t == 0), stop=(t == HT - 1))
                        for m in range(LT):
                            csb = stB.tile([128, TB], f32, tag="csb")
                            nc.scalar.copy(csb[:], dps[m][:])
                            nc.sync.dma_start(ckv_s[m, :, c0:c1], csb[:])
                        # rope the 64 k-rope rows, scatter into kT_full
                        rawk = stB.tile([KRR, TB], f32, tag="rawk")
                        nc.scalar.copy(rawk[:], krp[:])
                        rotk = stB.tile([KRR, TB], f32, tag="rotk")
                        nc.sync.dma_start(rotk[0:32, :], rawk[32:64, :])
                        nc.sync.dma_start(rotk[32:64, :], rawk[0:32, :])
                        ksb = stB.tile([KRR, TB], f32, tag="ksb")
                        nc.vector.tensor_mul(ksb[:], rawk[:], kcos_sb[:, c0:c1])
                        nc.vector.tensor_mul(rotk[:], rotk[:], ksin_sb[:, c0:c1])
                        nc.vector.tensor_add(ksb[:], ksb[:], rotk[:])
                        nc.sync.dma_start(kT_full[0:32, c0:c1], ksb[0:32, :].bitcast(F32R))
                        nc.sync.dma_start(kT_full[64:96, c0:c1], ksb[32:64, :].bitcast(F32R))

            # ---------------- phase 2: k_c and v from the latent ----------------
            with tc.tile_pool(name="wup", bufs=1) as wup, \
                 tc.tile_pool(name="ckvb", bufs=2) as ckvb, \
                 tc.tile_pool(name="st2", bufs=2) as st2, \
                 tc.tile_pool(name="psK", bufs=2, space=MS.PSUM) as psK, \
                 tc.tile_pool(name="psV", bufs=4, space=MS.PSUM) as psV:
                wupk_sb = wup.tile([128, LT, KRR], F32R, tag="upk")
                wupv_sb = wup.tile([128, LT, HEAD_DIM], F32R, tag="upv")
                nc.sync.dma_start(wupk_sb[:], wupk.rearrange("(t p) w -> p t w", p=128).bitcast(F32R))
                nc.sync.dma_start(wupv_sb[:], wupv.rearrange("(t p) w -> p t w", p=128).bitcast(F32R))
                for blk in range(NTB):
                    c0, c1 = blk * TB, (blk + 1) * TB
                    cb = ckvb.tile([128, LT, TB], F32R, tag="cb")
                    nc.sync.dma_start(cb[:], ckv_s[:, :, c0:c1].rearrange("t p w -> p t w").bitcast(F32R))
                    kcp = psK.tile([KRR, TB], f32, tag="kcp")
                    for lt in range(LT):
                        nc.tensor.matmul(kcp[:],
                                         wupk_sb[:, lt, :],
                                         cb[:, lt, :],
                                         start=(lt == 0), stop=(lt == LT - 1))
                    kcs = st2.tile([KRR, TB], f32, tag="kcs")
                    nc.scalar.copy(kcs[:], kcp[:])
                    nc.sync.dma_start(kT_full[32:64, c0:c1], kcs[0:32, :].bitcast(F32R))
                    nc.sync.dma_start(kT_full[96:128, c0:c1], kcs[32:64, :].bitcast(F32R))
                    for tt in range(TB // 128):
                        vp = psV.tile([128, HEAD_DIM], f32, tag="vp")
                        for lt in range(LT):
                            nc.tensor.matmul(
                                vp[:],
                                cb[:, lt, tt * 128:(tt + 1) * 128],
                                wupv_sb[:, lt, :],
                                start=(lt == 0), stop=(lt == LT - 1))
                        nc.scalar.copy(v_tok[:, blk * (TB // 128) + tt, :], vp[:])

            # ---------------- phases 3+4 ----------------
            with tc.tile_pool(name="attn", bufs=1) as ap_:
                attn_sb = ap_.tile([128, QT, NT], F32R)

                with tc.tile_pool(name="qh", bufs=2) as qhp, \
                     tc.tile_pool(name="cst3", bufs=1) as cst3, \
                     tc.tile_pool(name="pt", bufs=3) as ptp, \
                     tc.tile_pool(name="sm", bufs=2) as smp, \
                     tc.tile_pool(name="psS", bufs=3, space=MS.PSUM) as psS, \
                     tc.tile_pool(name="psO", bufs=2, space=MS.PSUM) as psO, \
                     tc.tile_pool(name="psU", bufs=2, space=MS.PSUM) as psU:
                    masks_sb = cst3.tile([128, NJ, QB], F32R, tag="masks")
                    nc.sync.dma_start(masks_sb[:], masks.bitcast(F32R))
                    ones_sb = cst3.tile([128, 1], F32R, tag="ones")
                    nc.sync.dma_start(ones_sb[:], onesd.bitcast(F32R))
                    for h in range(QT):
                        qh_sb = qhp.tile([128, NT], F32R, tag="qh")
                        nc.sync.dma_start(qh_sb[:], qT_s[h].bitcast(F32R))
                        for b in range(Bv):
                            off = b * Sv
                            for qb in range(NQB):
                                ops = psO.tile([128, QB], f32, tag="ops")
                                sps = psU.tile([1, QB], f32, tag="sps")
                                nkt = (qb + 1) * NJ
                                for kt in range(nkt):
                                    scp = psS.tile([128, QB], f32, tag="scp")
                                    nc.tensor.matmul(
                                        scp[:],
                                        kT_full[:, off + kt * 128: off + (kt + 1) * 128],
                                        qh_sb[:, off + qb * QB: off + (qb + 1) * QB],
                                        start=True, stop=True)
                                    ptile = ptp.tile([128, QB], F32R, tag="pt")
                                    nc.scalar.activation(ptile[:], scp[:], EXP)
                                    j = kt - qb * NJ
                                    if j >= 0:
                                        nc.vector.tensor_mul(ptile[:], ptile[:], masks_sb[:, j, :])
                                    nc.tensor.matmul(
                                        ops[:],
                                        v_tok[:, b * NKT_B + kt, :],
                                        ptile[:],
                                        start=(kt == 0), stop=(kt == nkt - 1))
                                    nc.tensor.matmul(
                                        sps[:],
                                        ones_sb[:],
                                        ptile[:],
                                        start=(kt == 0), stop=(kt == nkt - 1))
                                rec = smp.tile([1, QB], f32, tag="rec")
                                nc.vector.reciprocal(rec[:], sps[:])
                                rb = smp.tile([128, QB], f32, tag="rb")
                                nc.gpsimd.partition_broadcast(rb[:], rec[:])
                                nc.vector.tensor_mul(
                                    attn_sb[:, h, off + qb * QB: off + (qb + 1) * QB],
                                    ops[:], rb[:])

                # ---- phase 4: partial o_proj ----
                with tc.tile_pool(name="wop", bufs=1) as wop, \
                     tc.tile_pool(name="st4", bufs=4) as st4, \
                     tc.tile_pool(name="ps4", bufs=6, space=MS.PSUM) as ps4:
                    wo_sb = wop.tile([128, QT, HIDDEN], F32R)
                    nc.sync.dma_start(wo_sb[:], wo.rearrange("(t p) w -> p t w", p=128).bitcast(F32R))
                    for T in range(NT // 128):
                        for n in range(HIDDEN // 512):
                            ps = ps4.tile([128, 512], f32, tag="ps")
                            for h2 in range(QT):
                                nc.tensor.matmul(
                                    ps[:],
                                    attn_sb[:, h2, T * 128:(T + 1) * 128],
                                    wo_sb[:, h2, n * 512:(n + 1) * 512],
                                    start=(h2 == 0), stop=(h2 == QT - 1))
                            osb = st4.tile([128, 512], f32, tag="osb")
                            nc.vector.tensor_copy(osb[:], ps[:])
                            nc.sync.dma_start(outp[T * 128:(T + 1) * 128, n * 512:(n + 1) * 512], osb[:])

    nc.compile()
    return nc


def make_in_maps(hidden_states, Wq, Wkr, Wdk, Wupk, Wupv, Wo, Bv=B, Sv=S, QB=512):
    """Host-side sharding + layout prep. Returns per-core input dicts."""
    NT = Bv * Sv
    NJ = QB // 128
    scale = 1.0 / np.sqrt(np.float32(HEAD_DIM))

    hidT = np.ascontiguousarray(
        hidden_states.reshape(NT, HIDDEN).T.astype(np.float32))

    cos_t, sin_t = _rope_tables(Sv)                    # [128, S]
    cos_t = np.tile(cos_t, (1, Bv))                    # [128, NT]
    sin_t = np.tile(sin_t, (1, Bv))
    qcos = np.ascontiguousarray(cos_t * scale)
    qsin = np.ascontiguousarray(
        np.concatenate([-sin_t[0:64], sin_t[64:128]], axis=0) * scale)
    kcos = np.ascontiguousarray(
        np.concatenate([cos_t[0:32], cos_t[64:96]], axis=0))
    ksin = np.ascontiguousarray(
        np.concatenate([-sin_t[0:32], sin_t[64:96]], axis=0))

    k_idx = np.arange(128)[:, None]
    q_idx = np.arange(QB)[None, :]
    masks = np.stack(
        [(q_idx >= j * 128 + k_idx).astype(np.float32) for j in range(NJ)],
        axis=1)                                        # [128, NJ, QB]
    masks = np.ascontiguousarray(masks)

    in_maps = []
    for c in range(NCORES):
        wq_t = np.ascontiguousarray(Wq[QR * c:QR * (c + 1)].T.astype(np.float32))
        wkd_t = np.ascontiguousarray(
            np.concatenate([Wdk, Wkr[KRR * c:KRR * (c + 1)]], axis=0).T.astype(np.float32))
        wupk_t = np.ascontiguousarray(Wupk[KRR * c:KRR * (c + 1)].T.astype(np.float32))
        wupv_t = np.ascontiguousarray(
            Wupv[HEAD_DIM * c:HEAD_DIM * (c + 1)].T.astype(np.float32))
        wo_t = np.ascontiguousarray(Wo[:, QR * c:QR * (c + 1)].T.astype(np.float32))
        in_maps.append({
            "hidT": hidT, "wq_t": wq_t, "wkd_t": wkd_t,
            "wupk_t": wupk_t, "wupv_t": wupv_t, "wo_t": wo_t,
            "qcos": qcos, "qsin": qsin, "kcos": kcos, "ksin": ksin,
            "masks": masks, "ones": np.ones((128, 1), np.float32),
        })
    return in_maps


_NC_CACHE = {}


def _get_program(key=(B, S, 512, 512)):
    if key not in _NC_CACHE:
        _NC_CACHE[key] = build_program(*key)
    return _NC_CACHE[key]


def kernel(hidden_states, Wq, Wkr, Wdk, Wupk, Wupv, Wo):
    from concourse.bass_utils import run_bass_kernel_spmd

    hidden_states = np.asarray(hidden_states)
    in_maps = make_in_maps(hidden_states, np.asarray(Wq), np.asarray(Wkr),
                           np.asarray(Wdk), np.asarray(Wupk), np.asarray(Wupv),
                           np.asarray(Wo))
    nc = _get_program()
    res = run_bass_kernel_spmd(nc, in_maps, list(range(NCORES)))
    out = res.results[0]["out_part"].astype(np.float32)
    for i in range(1, NCORES):
        out = out + res.results[i]["out_part"]
    return out.reshape(B, S, HIDDEN).astype(np.float32)


# revision 5
# speedup vs baseline: 2.7286x; 2.7286x over previous
"""MLA (CustomLlamaMLAForInfer) Trainium2 Bass kernel.

Sharding: tensor-parallel over heads across 8 NeuronCores. Core c owns
kv-head c and q-heads [4c, 4c+4). Every core sees the full token stream
(B*S = 4096 tokens); o_proj is computed against the core's 512
head-dims, producing a partial [4096, 4096] output that the host sums
across the 8 cores.

Device program phases (single SPMD program, per-core weights differ):
  1a. qT = Wq_shard @ hidden.T   (rope + 1/sqrt(d) folded in at evict)
  1b. c_kvT = Wdk @ hidden.T ; krT = Wkr_shard @ hidden.T (rope at evict)
  2.  k_c / v from c_kvT via Wupk/Wupv shards; assemble kT_full, v_tok
  3.  causal attention per (batch, q-head): scores_T = kT.T@qT blocks,
      exp (no max-sub needed: |scores| < ~6), mask diag blocks,
      out_T[d,q] += v_tok.T @ p_T, sums via ones-matmul, normalize
  4.  partial o_proj: out[tok, hid] += attn_T.T @ WoT_shard

All matmuls run as float32r (fp22 mantissa, 1 PE pass).
"""

import numpy as np

HIDDEN = 4096
N_HEADS = 32
KV_HEADS = 8
HEAD_DIM = 128
LOW_RANK = 64
TOP_K_ROPE = 32
ROPE_THETA = 10000.0
B, S = 2, 2048
NCORES = 8
HPC = N_HEADS // NCORES          # q heads per core = 4
QR = HPC * HEAD_DIM              # q rows per core = 512
CD = LOW_RANK * KV_HEADS         # latent dim = 512
KRR = 2 * TOP_K_ROPE             # rope rows per kv head = 64


def _rope_tables(seq_len):
    inv = 1.0 / (ROPE_THETA ** (np.arange(0, HEAD_DIM, 2, dtype=np.float32) / HEAD_DIM))
    pos = np.arange(seq_len, dtype=np.float32)
    fr = np.outer(pos, inv)
    emb = np.concatenate([fr, fr], axis=-1)          # [S, 128]
    return (np.cos(emb).T.astype(np.float32),        # [128, S]
            np.sin(emb).T.astype(np.float32))


def build_program(Bv=B, Sv=S, TB=512, QB=512, trace_sim=False):
    """Build the SPMD Bass program. TB = proj token-block, QB = attention
    q-block (both <= 512, the fp32 moving-operand limit)."""
    from concourse import bacc, tile, mybir
    import concourse.bass as bass

    f32 = mybir.dt.float32
    F32R = mybir.dt.float32r
    MS = bass.MemorySpace
    EXP = mybir.ActivationFunctionType.Exp

    NT = Bv * Sv                 # total tokens
    HT = HIDDEN // 128           # hidden tiles = 32
    NTB = NT // TB               # proj token blocks
    NQB = Sv // QB               # q blocks per batch
    NJ = QB // 128               # diagonal mask variants
    NKT_B = Sv // 128            # k tiles per batch
    QT = QR // 128               # q-head tiles per core = 4
    LT = CD // 128               # latent tiles = 4

    nc = bacc.Bacc("TRN2", target_bir_lowering=False, debug=False,
                   num_devices=NCORES)

    def din(name, shape):
        return nc.dram_tensor(name, shape, f32, kind="ExternalInput").ap()

    hidT = din("hidT", [HIDDEN, NT])
    wq = din("wq_t", [HIDDEN, QR])
    wkd = din("wkd_t", [HIDDEN, CD + KRR])
    wupk = din("wupk_t", [CD, KRR])
    wupv = din("wupv_t", [CD, HEAD_DIM])
    wo = din("wo_t", [QR, HIDDEN])
    qcos = din("qcos", [128, NT])
    qsin = din("qsin", [128, NT])
    kcos = din("kcos", [KRR, NT])
    ksin = din("ksin", [KRR, NT])
    masks = din("masks", [128, NJ, QB])
    onesd = din("ones", [128, 1])
    outp = nc.dram_tensor("out_part", [NT, HIDDEN], f32, kind="ExternalOutput").ap()
    qT_s = nc.dram_tensor("qT_s", [QT, 128, NT], f32).ap()
    ckv_s = nc.dram_tensor("ckv_s", [LT, 128, NT], f32).ap()

    with tile.TileContext(nc, trace_sim=trace_sim) as tc:
        with tc.tile_pool(name="persist", bufs=1) as pers:
            kT_full = pers.tile([128, NT], F32R, tag="kT")
            v_tok = pers.tile([128, NT // 128, HEAD_DIM], F32R, tag="vtok")

            # ---------------- phase 1: projections of hidden ----------------
            with tc.tile_pool(name="cos", bufs=1) as cp:
                qcos_sb = cp.tile([128, NT], f32, tag="qc")
                qsin_sb = cp.tile([128, NT], f32, tag="qs")
                kcos_sb = cp.tile([KRR, NT], f32, tag="kc")
                ksin_sb = cp.tile([KRR, NT], f32, tag="ks")
                nc.sync.dma_start(qcos_sb[:], qcos)
                nc.sync.dma_start(qsin_sb[:], qsin)
                nc.sync.dma_start(kcos_sb[:], kcos)
                nc.sync.dma_start(ksin_sb[:], ksin)

                # ---- pass A: q projection (+rope, +1/sqrt(d) via tables) ----
                with tc.tile_pool(name="wqp", bufs=1) as wqp, \
                     tc.tile_pool(name="hidA", bufs=8) as hpA, \
                     tc.tile_pool(name="stA", bufs=2) as stA, \
                     tc.tile_pool(name="psA", bufs=8, space=MS.PSUM) as ppA:
                    wq_sb = wqp.tile([128, HT, QR], F32R)
                    nc.sync.dma_start(wq_sb[:], wq.rearrange("(t p) w -> p t w", p=128).bitcast(F32R))
                    for blk in range(NTB):
                        c0, c1 = blk * TB, (blk + 1) * TB
                        qps = [ppA.tile([128, TB], f32, tag="qps", name=f"qps{_m}") for _m in range(QT)]
                        for t in range(HT):
                            ht = hpA.tile([128, TB], F32R, tag="hid")
                            nc.sync.dma_start(ht[:], hidT[t * 128:(t + 1) * 128, c0:c1].bitcast(F32R))
                            for m in range(QT):
                                nc.tensor.matmul(
                                    qps[m][:],
                                    wq_sb[:, t, m * 128:(m + 1) * 128],
                                    ht[:],
                                    start=(t == 0), stop=(t == HT - 1))
                        for m in range(QT):
                            raw = stA.tile([128, TB], f32, tag="raw")
                            nc.scalar.copy(raw[:], qps[m][:])
                            rot = stA.tile([128, TB], f32, tag="rot")
                            nc.sync.dma_start(rot[0:64, :], raw[64:128, :])
                            nc.sync.dma_start(rot[64:128, :], raw[0:64, :])
                            qsb = stA.tile([128, TB], f32, tag="qsb")
                            nc.vector.tensor_mul(qsb[:], raw[:], qcos_sb[:, c0:c1])
                            nc.vector.tensor_mul(rot[:], rot[:], qsin_sb[:, c0:c1])
                            nc.vector.tensor_add(qsb[:], qsb[:], rot[:])
                            nc.sync.dma_start(qT_s[m, :, c0:c1], qsb[:])

                # ---- pass B: c_kv (latent) + k_rope projections ----
                with tc.tile_pool(name="wkdp", bufs=1) as wkdp, \
                     tc.tile_pool(name="hidB", bufs=8) as hpB, \
                     tc.tile_pool(name="stB", bufs=2) as stB, \
                     tc.tile_pool(name="psB", bufs=6, space=MS.PSUM) as ppB, \
                     tc.tile_pool(name="psBk", bufs=2, space=MS.PSUM) as ppBk:
                    wkd_sb = wkdp.tile([128, HT, CD + KRR], F32R)
                    nc.sync.dma_start(wkd_sb[:], wkd.rearrange("(t p) w -> p t w", p=128).bitcast(F32R))
                    for blk in range(NTB):
                        c0, c1 = blk * TB, (blk + 1) * TB
                        dps = [ppB.tile([128, TB], f32, tag="dps", name=f"dps{_m}") for _m in range(LT)]
                        krp = ppBk.tile([KRR, TB], f32, tag="krp")
                        for t in range(HT):
                            ht = hpB.tile([128, TB], F32R, tag="hid")
                            nc.sync.dma_start(ht[:], hidT[t * 128:(t + 1) * 128, c0:c1].bitcast(F32R))
                            for m in range(LT):
                                nc.tensor.matmul(
                                    dps[m][:],
                                    wkd_sb[:, t, m * 128:(m + 1) * 128],
                                    ht[:],
                                    start=(t == 0), stop=(t == HT - 1))
                            nc.tensor.matmul(
                                krp[:],
                                wkd_sb[:, t, CD:CD + KRR],
                                ht[:],
                                start=(t == 0), stop=(t == HT - 1))
                        for m in range(LT):
                            csb = stB.tile([128, TB], f32, tag="csb")
                            nc.scalar.copy(csb[:], dps[m][:])
                            nc.sync.dma_start(ckv_s[m, :, c0:c1], csb[:])
                        # rope the 64 k-rope rows, scatter into kT_full
                        rawk = stB.tile([KRR, TB], f32, tag="rawk")
                        nc.scalar.copy(rawk[:], krp[:])
                        rotk = stB.tile([KRR, TB], f32, tag="rotk")
                        nc.sync.dma_start(rotk[0:32, :], rawk[32:64, :])
                        nc.sync.dma_start(rotk[32:64, :], rawk[0:32, :])
                        ksb = stB.tile([KRR, TB], f32, tag="ksb")
                        nc.vector.tensor_mul(ksb[:], rawk[:], kcos_sb[:, c0:c1])
                        nc.vector.tensor_mul(rotk[:], rotk[:], ksin_sb[:, c0:c1])
                        nc.vector.tensor_add(ksb[:], ksb[:], rotk[:])
                        nc.sync.dma_start(kT_full[0:32, c0:c1], ksb[0:32, :].bitcast(F32R))
                        nc.sync.dma_start(kT_full[64:96, c0:c1], ksb[32:64, :].bitcast(F32R))

            # ---------------- phase 2: k_c and v from the latent ----------------
            with tc.tile_pool(name="wup", bufs=1) as wup, \
                 tc.tile_pool(name="ckvb", bufs=2) as ckvb, \
                 tc.tile_pool(name="st2", bufs=2) as st2, \
                 tc.tile_pool(name="psK", bufs=2, space=MS.PSUM) as psK, \
                 tc.tile_pool(name="psV", bufs=4, space=MS.PSUM) as psV:
                wupk_sb = wup.tile([128, LT, KRR], F32R, tag="upk")
                wupv_sb = wup.tile([128, LT, HEAD_DIM], F32R, tag="upv")
                nc.sync.dma_start(wupk_sb[:], wupk.rearrange("(t p) w -> p t w", p=128).bitcast(F32R))
                nc.sync.dma_start(wupv_sb[:], wupv.rearrange("(t p) w -> p t w", p=128).bitcast(F32R))
                for blk in range(NTB):
                    c0, c1 = blk * TB, (blk + 1) * TB
                    cb = ckvb.tile([128, LT, TB], F32R, tag="cb")
                    nc.sync.dma_start(cb[:], ckv_s[:, :, c0:c1].rearrange("t p w -> p t w").bitcast(F32R))
                    kcp = psK.tile([KRR, TB], f32, tag="kcp")
                    for lt in range(LT):
                        nc.tensor.matmul(kcp[:],
                                         wupk_sb[:, lt, :],
                                         cb[:, lt, :],
                                         start=(lt == 0), stop=(lt == LT - 1))
                    kcs = st2.tile([KRR, TB], f32, tag="kcs")
                    nc.scalar.copy(kcs[:], kcp[:])
                    nc.sync.dma_start(kT_full[32:64, c0:c1], kcs[0:32, :].bitcast(F32R))
                    nc.sync.dma_start(kT_full[96:128, c0:c1], kcs[32:64, :].bitcast(F32R))
                    for tt in range(TB // 128):
                        vp = psV.tile([128, HEAD_DIM], f32, tag="vp")
                        for lt in range(LT):
                            nc.tensor.matmul(
                                vp[:],
                                cb[:, lt, tt * 128:(tt + 1) * 128],
                                wupv_sb[:, lt, :],
                                start=(lt == 0), stop=(lt == LT - 1))
                        nc.scalar.copy(v_tok[:, blk * (TB // 128) + tt, :], vp[:])

            # ---------------- phases 3+4 ----------------
            with tc.tile_pool(name="attn", bufs=1) as ap_:
                attn_sb = ap_.tile([128, QT, NT], F32R)

                with tc.tile_pool(name="qh", bufs=2) as qhp, \
                     tc.tile_pool(name="cst3", bufs=1) as cst3, \
                     tc.tile_pool(name="pt", bufs=3) as ptp, \
                     tc.tile_pool(name="sm", bufs=2) as smp, \
                     tc.tile_pool(name="psS", bufs=3, space=MS.PSUM) as psS, \
                     tc.tile_pool(name="psO", bufs=2, space=MS.PSUM) as psO, \
                     tc.tile_pool(name="psU", bufs=2, space=MS.PSUM) as psU:
                    masks_sb = cst3.tile([128, NJ, QB], F32R, tag="masks")
                    nc.sync.dma_start(masks_sb[:], masks.bitcast(F32R))
                    ones_sb = cst3.tile([128, 1], F32R, tag="ones")
                    nc.sync.dma_start(ones_sb[:], onesd.bitcast(F32R))
                    for h in range(QT):
                        qh_sb = qhp.tile([128, NT], F32R, tag="qh")
                        nc.sync.dma_start(qh_sb[:], qT_s[h].bitcast(F32R))
                        for b in range(Bv):
                            off = b * Sv
                            for qb in range(NQB):
                                ops = psO.tile([128, QB], f32, tag="ops")
                                sps = psU.tile([1, QB], f32, tag="sps")
                                nkt = (qb + 1) * NJ
                                for kt in range(nkt):
                                    scp = psS.tile([128, QB], f32, tag="scp")
                                    nc.tensor.matmul(
                                        scp[:],
                                        kT_full[:, off + kt * 128: off + (kt + 1) * 128],
                                        qh_sb[:, off + qb * QB: off + (qb + 1) * QB],
                                        start=True, stop=True)
                                    ptile = ptp.tile([128, QB], F32R, tag="pt")
                                    nc.scalar.activation(ptile[:], scp[:], EXP)
                                    j = kt - qb * NJ
                                    if j >= 0:
                                        nc.vector.tensor_mul(ptile[:], ptile[:], masks_sb[:, j, :])
                                    nc.tensor.matmul(
                                        ops[:],
                                        v_tok[:, b * NKT_B + kt, :],
                                        ptile[:],
                                        start=(kt == 0), stop=(kt == nkt - 1))
                                    nc.tensor.matmul(
                                        sps[:],
                                        ones_sb[:],
                                        ptile[:],
                                        start=(kt == 0), stop=(kt == nkt - 1))
                                rec = smp.tile([1, QB], f32, tag="rec")
                                nc.vector.reciprocal(rec[:], sps[:])
                                rb = smp.tile([128, QB], f32, tag="rb")
                                nc.gpsimd.partition_broadcast(rb[:], rec[:])
                                nc.vector.tensor_mul(
                                    attn_sb[:, h, off + qb * QB: off + (qb + 1) * QB],
                                    ops[:], rb[:])

                # ---- phase 4: partial o_proj ----
                with tc.tile_pool(name="wop", bufs=1) as wop, \
                     tc.tile_pool(name="st4", bufs=4) as st4, \
                     tc.tile_pool(name="ps4", bufs=6, space=MS.PSUM) as ps4:
                    wo_sb = wop.tile([128, QT, HIDDEN], F32R)
                    nc.sync.dma_start(wo_sb[:], wo.rearrange("(t p) w -> p t w", p=128).bitcast(F32R))
                    for T in range(NT // 128):
                        for n in range(HIDDEN // 512):
                            ps = ps4.tile([128, 512], f32, tag="ps")
                            for h2 in range(QT):
                                nc.tensor.matmul(
                                    ps[:],
                                    attn_sb[:, h2, T * 128:(T + 1) * 128],
                                    wo_sb[:, h2, n * 512:(n + 1) * 512],
                                    start=(h2 == 0), stop=(h2 == QT - 1))
                            osb = st4.tile([128, 512], f32, tag="osb")
                            nc.vector.tensor_copy(osb[:], ps[:])
                            nc.sync.dma_start(outp[T * 128:(T + 1) * 128, n * 512:(n + 1) * 512], osb[:])

    nc.compile()
    return nc


def make_in_maps(hidden_states, Wq, Wkr, Wdk, Wupk, Wupv, Wo, Bv=B, Sv=S, QB=512):
    """Host-side sharding + layout prep. Returns per-core input dicts."""
    NT = Bv * Sv
    NJ = QB // 128
    scale = 1.0 / np.sqrt(np.float32(HEAD_DIM))

    hidT = np.ascontiguousarray(
        hidden_states.reshape(NT, HIDDEN).T.astype(np.float32))

    cos_t, sin_t = _rope_tables(Sv)                    # [128, S]
    cos_t = np.tile(cos_t, (1, Bv))                    # [128, NT]
    sin_t = np.tile(sin_t, (1, Bv))
    qcos = np.ascontiguousarray(cos_t * scale)
    qsin = np.ascontiguousarray(
        np.concatenate([-sin_t[0:64], sin_t[64:128]], axis=0) * scale)
    kcos = np.ascontiguousarray(
        np.concatenate([cos_t[0:32], cos_t[64:96]], axis=0))
    ksin = np.ascontiguousarray(
        np.concatenate([-sin_t[0:32], sin_t[64:96]], axis=0))

    k_idx = np.arange(128)[:, None]
    q_idx = np.arange(QB)[None, :]
    masks = np.stack(
        [(q_idx >= j * 128 + k_idx).astype(np.float32) for j in range(NJ)],
        axis=1)                                        # [128, NJ, QB]
    masks = np.ascontiguousarray(masks)

    in_maps = []
    for c in range(NCORES):
        wq_t = np.ascontiguousarray(Wq[QR * c:QR * (c + 1)].T.astype(np.float32))
        wkd_t = np.ascontiguousarray(
            np.concatenate([Wdk, Wkr[KRR * c:KRR * (c + 1)]], axis=0).T.astype(np.float32))
        wupk_t = np.ascontiguousarray(Wupk[KRR * c:KRR * (c + 1)].T.astype(np.float32))
        wupv_t = np.ascontiguousarray(
            Wupv[HEAD_DIM * c:HEAD_DIM * (c + 1)].T.astype(np.float32))
        wo_t = np.ascontiguousarray(Wo[:, QR * c:QR * (c + 1)].T.astype(np.float32))
        in_maps.append({
            "hidT": hidT, "wq_t": wq_t, "wkd_t": wkd_t,
            "wupk_t": wupk_t, "wupv_t": wupv_t, "wo_t": wo_t,
            "qcos": qcos, "qsin": qsin, "kcos": kcos, "ksin": ksin,
            "masks": masks, "ones": np.ones((128, 1), np.float32),
        })
    return in_maps


_NC_CACHE = {}


def _get_program(key=(B, S, 512, 512)):
    if key not in _NC_CACHE:
        _NC_CACHE[key] = build_program(*key)
    return _NC_CACHE[key]


def kernel(hidden_states, Wq, Wkr, Wdk, Wupk, Wupv, Wo):
    from concourse.bass_utils import run_bass_kernel_spmd

    hidden_states = np.asarray(hidden_states)
    in_maps = make_in_maps(hidden_states, np.asarray(Wq), np.asarray(Wkr),
                           np.asarray(Wdk), np.asarray(Wupk), np.asarray(Wupv),
                           np.asarray(Wo))
    nc = _get_program()
    res = run_bass_kernel_spmd(nc, in_maps, list(range(NCORES)))
    out = res.results[0]["out_part"].astype(np.float32)
    for i in range(1, NCORES):
        out = out + res.results[i]["out_part"]
    return out.reshape(B, S, HIDDEN).astype(np.float32)
